# revision 1
# baseline (speedup 1.0000x reference)
"""GSAT graph-attention kernel for 8 Trainium2 NeuronCores.

Math (per batch b):
  h = x @ W                                     [N, 512]
  ss[i] = h[i] . a_src_flat / H ; sd[j] = h[j] . a_dst_flat / H
  t[i,j] = (ss[i] + sd[j]) * adj[i,j] + gumbel(noise[b,i,j])
  A1 = softmax_j(t) ; A2 = softmax_j(A1)
  out[b,n] = sum_i A2[i,n] * h[i] @ W_out

Sharding: 8 cores = (batch b in 0..3) x (row-half rb in 0..1).  Rows i are
sharded; both softmaxes are along j (within-row), so each core computes its
2048 rows completely and produces a partial output  outT = W_out^T h'^T
summed over its rows; host adds the two row-half partials per batch.

Device-side structure per core:
  phase 0: hT = (x W)^T for own rows, s-vectors via folded W@a weights,
           k = h @ W_out  (W_out folded BEFORE aggregation: (A^T H)Wo = A^T(H Wo))
  main:    per 128-row block: gumbel + scores via rank-2 PE matmul, two
           softmax passes on ACT (row sums via accum_out), then 8 matmuls
           k~^T @ e2 accumulated in PSUM across all 16 blocks.
  epilog:  copy the 8 PSUM accumulators out.

Normalizations are folded: 1/rowsum1 into the second Exp's per-partition
scale, 1/rowsum2 into k (k~ = k/rs2), so unnormalized e2 feeds the matmul.
"""

import os
import sys

for _p in ("/opt/trn_rl_repo",):
    if _p not in sys.path and os.path.isdir(_p):
        sys.path.insert(0, _p)

os.environ.setdefault("MYCRO_LOCAL_CACHE", "1")

import numpy as np
import ml_dtypes

B, N, IN_F, H, OUT_F = 4, 4096, 256, 8, 64
D = H * OUT_F          # 512
RB = N // 2            # 2048 rows per core
NBLK = RB // 128       # 16 row blocks per core
EPS = 1e-9
N_CORES = 8

_cache = {}


def _build_module():
    import concourse.bacc as bacc
    import concourse.tile as tile
    from concourse import mybir

    f32 = mybir.dt.float32
    f32r = mybir.dt.float32r
    bf16 = mybir.dt.bfloat16
    AF = mybir.ActivationFunctionType
    ALU = mybir.AluOpType

    nc = bacc.Bacc("TRN2", target_bir_lowering=False)

    xT_d = nc.declare_dram_parameter("xT", [IN_F, N], f32r, isOutput=False)
    xTr_d = nc.declare_dram_parameter("xTr", [IN_F, RB], f32r, isOutput=False)
    adj_d = nc.declare_dram_parameter("adj_s", [RB, N], bf16, isOutput=False)
    nz_d = nc.declare_dram_parameter("noise_s", [RB, N], f32, isOutput=False)
    W_d = nc.declare_dram_parameter("W", [IN_F, D], f32r, isOutput=False)
    wsd_d = nc.declare_dram_parameter("wsd", [IN_F, 2], f32r, isOutput=False)
    Wo_d = nc.declare_dram_parameter("W_out", [D, OUT_F], f32r, isOutput=False)
    outT_d = nc.declare_dram_parameter("outT", [OUT_F, N], f32, isOutput=True)

    with tile.TileContext(nc) as tc:
        import contextlib

        with contextlib.ExitStack() as ctx:
            pers = ctx.enter_context(tc.tile_pool(name="pers", bufs=1))
            # persistent small tensors
            sdb = pers.tile([128, N], f32)      # s_dst broadcast down partitions
            ss_col = pers.tile([128, NBLK], f32)  # ss_col[p, b] = s_src[b*128+p]
            ktil = [pers.tile([128, OUT_F], f32r, tag=f"k{ib}", name=f"k{ib}") for ib in range(NBLK)]

            epsb = pers.tile([128, 1], f32)
            nc.vector.memset(epsb, EPS)

            # ---------------- phase 0 ----------------
            with tc.tile_pool(name="p0", bufs=1) as p0, \
                 tc.tile_pool(name="ps0", bufs=2, space="PSUM") as ps0:
                xT2 = [p0.tile([128, N], f32r, tag=f"xT{fc}", name=f"xT{fc}") for fc in range(2)]
                xTr2 = [p0.tile([128, RB], f32r, tag=f"xTr{fc}", name=f"xTr{fc}") for fc in range(2)]
                Wt = [p0.tile([128, D], f32r, tag=f"W{fc}", name=f"Wti{fc}") for fc in range(2)]
                wsdt = [p0.tile([128, 2], f32r, tag=f"wsd{fc}", name=f"wsdt{fc}") for fc in range(2)]
                Wot = [p0.tile([128, OUT_F], f32r, tag=f"Wo{dc}", name=f"Wot{dc}") for dc in range(4)]
                for fc in range(2):
                    nc.sync.dma_start(out=xT2[fc], in_=xT_d[fc * 128:(fc + 1) * 128, :])
                    nc.sync.dma_start(out=xTr2[fc], in_=xTr_d[fc * 128:(fc + 1) * 128, :])
                    nc.sync.dma_start(out=Wt[fc], in_=W_d[fc * 128:(fc + 1) * 128, :])
                    nc.sync.dma_start(out=wsdt[fc], in_=wsd_d[fc * 128:(fc + 1) * 128, :])
                for dc in range(4):
                    nc.sync.dma_start(out=Wot[dc], in_=Wo_d[dc * 128:(dc + 1) * 128, :])

                # s_dst row [1, N] then broadcast down 128 partitions
                sd_row = p0.tile([1, N], f32)
                for jc in range(8):
                    sps = ps0.tile([1, 512], f32, tag="sps")
                    for fc in range(2):
                        nc.tensor.matmul(sps, wsdt[fc][:, 1:2].bitcast(f32),
                                         xT2[fc][:, jc * 512:(jc + 1) * 512].bitcast(f32),
                                         start=(fc == 0), stop=(fc == 1))
                    nc.vector.tensor_copy(sd_row[0:1, jc * 512:(jc + 1) * 512], sps)
                sd_dram = nc.dram_tensor("sd_scratch", [1, N], f32)
                nc.sync.dma_start(out=sd_dram[:], in_=sd_row)
                import concourse.bass as bass_mod
                sd_bcast = bass_mod.AP(tensor=sd_dram[:].tensor,
                                       offset=sd_dram[:].offset,
                                       ap=[[0, 128]] + list(sd_dram[:].ap)[1:])
                nc.gpsimd.dma_start(out=sdb, in_=sd_bcast)
                # ss_col[p, b] = s_src of row b*128+p (one N=1 matmul per block)
                sscol_ps = ps0.tile([128, NBLK], f32, tag="sscol")
                for ib in range(NBLK):
                    for fc in range(2):
                        nc.tensor.matmul(sscol_ps[:, ib:ib + 1],
                                         xTr2[fc][:, ib * 128:(ib + 1) * 128].bitcast(f32),
                                         wsdt[fc][:, 0:1].bitcast(f32),
                                         start=(fc == 0), stop=(fc == 1))
                nc.vector.tensor_copy(ss_col, sscol_ps)

                # hT[dc][d, i] = h[i, d] for own rows; then k = h @ W_out
                with tc.tile_pool(name="hp", bufs=1) as hp:
                    hT = [hp.tile([128, RB], f32r, tag=f"hT{dc}", name=f"hT{dc}") for dc in range(4)]
                    for dc in range(4):
                        for ic in range(RB // 512):
                            hps = ps0.tile([128, 512], f32, tag="hps")
                            for fc in range(2):
                                nc.tensor.matmul(
                                    hps,
                                    Wt[fc][:, dc * 128:(dc + 1) * 128],
                                    xTr2[fc][:, ic * 512:(ic + 1) * 512],
                                    start=(fc == 0), stop=(fc == 1))
                            nc.vector.tensor_copy(hT[dc][:, ic * 512:(ic + 1) * 512], hps)
                    for ib in range(NBLK):
                        kps = ps0.tile([128, OUT_F], f32, tag="kps")
                        for dc in range(4):
                            nc.tensor.matmul(kps,
                                             hT[dc][:, ib * 128:(ib + 1) * 128].bitcast(f32),
                                             Wot[dc].bitcast(f32),
                                             start=(dc == 0), stop=(dc == 3))
                        nc.vector.tensor_copy(ktil[ib], kps)

            # ---------------- main loop ----------------
            with tc.tile_pool(name="agg", bufs=1, space="PSUM") as aggpool, \
                 tc.tile_pool(name="stream", bufs=3) as spool, \
                 tc.tile_pool(name="smalls", bufs=4) as rpool:
                aggp = [aggpool.tile([64, 512], f32, tag=f"agg{j}", name=f"agg{j}") for j in range(8)]

                # stage A: DMA noise/adj + gumbel Ln passes + score stt,
                # emitted one block AHEAD of stage B so ACT never stalls on DVE.
                def stage_a(ib):
                    nz = spool.tile([128, N], f32, tag="nz", name=f"nz{ib}")
                    nc.sync.dma_start(out=nz, in_=nz_d[ib * 128:(ib + 1) * 128, :])
                    ad = spool.tile([128, N], bf16, tag="ad", name=f"ad{ib}")
                    nc.sync.dma_start(out=ad, in_=adj_d[ib * 128:(ib + 1) * 128, :])
                    m = spool.tile([128, N], f32r, tag="m", name=f"m{ib}")
                    # v = log(noise + EPS); g = log(EPS - v)   (in-place)
                    nc.scalar.activation(out=nz, in_=nz, func=AF.Ln, bias=epsb, scale=1.0)
                    nc.scalar.activation(out=nz, in_=nz, func=AF.Ln, bias=epsb, scale=-1.0)
                    # m = (sd[j] + ss[i]) * adj   (one fused stt)
                    nc.vector.scalar_tensor_tensor(out=m, in0=sdb,
                                                   scalar=ss_col[:, ib:ib + 1],
                                                   in1=ad, op0=ALU.add, op1=ALU.mult)
                    return nz, m

                staged = {0: stage_a(0)}
                for ib in range(NBLK):
                    if ib + 1 < NBLK:
                        staged[ib + 1] = stage_a(ib + 1)
                    nz, m = staged.pop(ib)
                    # t = m - g  (computed as (g * -1) + m)
                    nc.vector.scalar_tensor_tensor(out=m, in0=nz, scalar=-1.0, in1=m,
                                                   op0=ALU.mult, op1=ALU.add)

                    # e1 = exp(t), rs1 = rowsum(e1)
                    rs1 = rpool.tile([128, 1], f32, tag="rs1")
                    nc.scalar.activation(out=m, in_=m, func=AF.Exp, accum_out=rs1)
                    rs1r = rpool.tile([128, 1], f32, tag="rs1r")
                    nc.vector.reciprocal(rs1r, rs1)

                    # e2 = exp(e1/rs1), rs2 = rowsum(e2)
                    rs2 = rpool.tile([128, 1], f32, tag="rs2")
                    nc.scalar.activation(out=m, in_=m, func=AF.Exp, scale=rs1r,
                                         accum_out=rs2)
                    rs2r = rpool.tile([128, 1], f32, tag="rs2r")
                    nc.vector.reciprocal(rs2r, rs2)

                    # k~ = k / rs2
                    kt = rpool.tile([128, OUT_F], f32r, tag="kt")
                    nc.vector.tensor_scalar(out=kt, in0=ktil[ib], scalar1=rs2r,
                                            scalar2=None, op0=ALU.mult)

                    # outT += k~^T @ e2 : accumulate in PSUM across all blocks
                    for ns in range(8):
                        nc.tensor.matmul(aggp[ns], kt,
                                         m[:, ns * 512:(ns + 1) * 512],
                                         start=(ib == 0), stop=(ib == NBLK - 1))

                # ---------------- epilogue ----------------
                with tc.tile_pool(name="fin", bufs=1) as fpool:
                    outT = fpool.tile([OUT_F, N], f32)
                    for ns in range(8):
                        nc.vector.tensor_copy(
                            outT[:, ns * 512:(ns + 1) * 512], aggp[ns])
                    nc.sync.dma_start(out=outT_d[:], in_=outT)

    nc.compile()
    return nc


def _get_module():
    if "nc" not in _cache:
        _cache["nc"] = _build_module()
    return _cache["nc"]


def kernel(x, adj, noise, W, a_src, a_dst, W_out):
    from concourse.bass_utils import run_bass_kernel_spmd

    nc = _get_module()

    x = np.asarray(x, dtype=np.float32)
    adj = np.asarray(adj, dtype=np.float32)
    noise = np.asarray(noise, dtype=np.float32)
    W = np.asarray(W, dtype=np.float32)
    a_src = np.asarray(a_src, dtype=np.float32)
    a_dst = np.asarray(a_dst, dtype=np.float32)
    W_out = np.asarray(W_out, dtype=np.float32)

    # fold the per-head score weights: s = (x @ W) @ a_flat / H == x @ (W @ a_flat / H)
    w_src = (W @ a_src.reshape(-1)) / H
    w_dst = (W @ a_dst.reshape(-1)) / H
    wsd = np.ascontiguousarray(np.stack([w_src, w_dst], axis=1), dtype=np.float32)
    adj_bf = adj.astype(ml_dtypes.bfloat16)  # exact for 0/1 values
    Wc = np.ascontiguousarray(W)
    Woc = np.ascontiguousarray(W_out)

    in_maps = []
    for core in range(N_CORES):
        b, rb = core // 2, core % 2
        rows = slice(rb * RB, (rb + 1) * RB)
        xTb = np.ascontiguousarray(x[b].T)  # [IN_F, N]
        in_maps.append({
            "xT": xTb,
            "xTr": np.ascontiguousarray(xTb[:, rows]),
            "adj_s": np.ascontiguousarray(adj_bf[rows, :]),
            "noise_s": np.ascontiguousarray(noise[b, rows, :]),
            "W": Wc,
            "wsd": wsd,
            "W_out": Woc,
        })

    res = run_bass_kernel_spmd(nc, in_maps, list(range(N_CORES)))
    kernel._last_results = res

    out = np.empty((B, N, OUT_F), dtype=np.float32)
    for b in range(B):
        acc = res.results[2 * b]["outT"].astype(np.float32) + \
            res.results[2 * b + 1]["outT"].astype(np.float32)
        out[b] = acc.T
    return out



# revision 11
# speedup vs baseline: 1.2430x; 1.2430x over previous
"""GSAT graph-attention kernel for 8 Trainium2 NeuronCores.

Math (per batch b):
  h = x @ W                                     [N, 512]
  ss[i] = h[i] . a_src_flat / H ; sd[j] = h[j] . a_dst_flat / H
  t[i,j] = (ss[i] + sd[j]) * adj[i,j] + gumbel(noise[b,i,j])
  A1 = softmax_j(t) ; A2 = softmax_j(A1)
  out[b,n] = sum_i A2[i,n] * (h[i] @ W_out)

Sharding: 8 cores = (batch b in 0..3) x (row-half rb in 0..1).  Rows i are
sharded; both softmaxes are along j (within-row), so each core computes its
2048 rows completely and produces a partial output outT summed over its
rows; host adds the two row-half partials per batch.

Engine budget note: the elementwise chain is 3 transcendentals + arith per
element.  ACT runs ~1 elem/cycle dtype-independent; DVE runs 2x on all-bf16
tensor ops.  Two per-block schedules are mixed to balance ACT vs DVE:

  route A (ACT-heavy, 4 ACT passes):
     v  = Ln(1 - w)            [ACT, bf16]        (w = 1-u host-encoded)
     g' = Ln(eps - v)          [ACT, bf16]        (= -gumbel)
     m  = (sd_j + ss_i)*adj    [DVE stt, bf16 2x]
     t  = m - g'               [DVE stt, bf16 2x]
     e1 = Exp(t)        accum rs1   [ACT]
     e2 = Exp(e1/rs1)   accum rs2   [ACT]

  route B (DVE-heavy, 2 ACT passes), using
  exp(scores+gumbel) = (1 + adj*(exp(ss_i)exp(sd_j) - 1)) * 1/(-ln(u)):
     v  = Ln(1 - w)            [ACT, f32 out]
     r  = 1/v                  [DVE reciprocal_approx_fast]  (NEGATIVE)
     w2 = esd_j*ess_i - 1      [DVE ts, bf16 4x]
     p  = w2 * adj             [GPSIMD tt (offload) or DVE tt bf16 2x]
     e1 = (p + 1) * r   accum rs1   [DVE stt]    (e1, rs1 both negative;
     e2 = Exp(e1/rs1)   accum rs2   [ACT]         the sign cancels in e1/rs1)

W_out is folded before aggregation ((A^T H)Wo = A^T(H Wo)); k = x @ (W Wo)
with W Wo folded on the host.  1/rs2 is folded into k.  Both Ln and Exp live
in the 'natural_log_exp_and_others' ACT table set; compile-time table lists
are filtered so the fixpoint pass picks that set (1 table load instead of the
per-block ping-pong between natural_log and exp_and_others).
"""

import os
import sys

for _p in ("/opt/trn_rl_repo",):
    if _p not in sys.path and os.path.isdir(_p):
        sys.path.insert(0, _p)

os.environ.setdefault("MYCRO_LOCAL_CACHE", "1")

import numpy as np
import ml_dtypes

B, N, IN_F, H, OUT_F = 4, 4096, 256, 8, 64
D = H * OUT_F          # 512
RB = N // 2            # 2048 rows per core
NBLK = RB // 128       # 16 row blocks per core
EPS = 1e-9
N_CORES = 8

# Tuning knobs (env overrides are for local experiments only; defaults baked)
ROUTE_A = set(int(x) for x in os.environ.get("KRN_ROUTE_A", "2,6,10,14").split(",") if x != "")
GP_TT = os.environ.get("KRN_GP_TT", "1") == "1"   # B-route mask-mult on GPSIMD

_cache = {}


def _build_module():
    import contextlib

    import concourse.bacc as bacc
    import concourse.bass as bass_mod
    import concourse.tile as tile
    from concourse import mybir

    f32 = mybir.dt.float32
    bf16 = mybir.dt.bfloat16
    AF = mybir.ActivationFunctionType
    ALU = mybir.AluOpType

    nc = bacc.Bacc("TRN2", target_bir_lowering=False)

    xT_d = nc.declare_dram_parameter("xT", [IN_F, N], bf16, isOutput=False)
    xTr_d = nc.declare_dram_parameter("xTr", [IN_F, RB], bf16, isOutput=False)
    adj_d = nc.declare_dram_parameter("adj_s", [RB, N], bf16, isOutput=False)
    wn_d = nc.declare_dram_parameter("wn_s", [RB, N], bf16, isOutput=False)
    wsd_d = nc.declare_dram_parameter("wsd", [IN_F, 2], bf16, isOutput=False)
    WWo_d = nc.declare_dram_parameter("WWo", [IN_F, OUT_F], bf16, isOutput=False)
    outT_d = nc.declare_dram_parameter("outT", [OUT_F, N], f32, isOutput=True)

    with tile.TileContext(nc) as tc:
        with contextlib.ExitStack() as ctx:
            pers = ctx.enter_context(tc.tile_pool(name="pers", bufs=1))
            sdb = pers.tile([128, N], bf16)       # raw s_dst broadcast (route A)
            sdbe = pers.tile([128, N], bf16)      # exp(s_dst) broadcast (route B)
            ss_col = pers.tile([128, NBLK], f32)  # ss_col[p, b] = s_src[b*128+p]
            ess_col = pers.tile([128, NBLK], f32)  # exp(ss_col)
            ktil = [pers.tile([128, OUT_F], f32, tag=f"k{ib}", name=f"k{ib}")
                    for ib in range(NBLK)]
            sd_rows = pers.tile([1, 2 * N], bf16)  # [raw sd | exp sd] rows
            epsb = pers.tile([128, 1], f32)
            nc.vector.memset(epsb, EPS)
            oneb = pers.tile([128, 1], f32)
            nc.vector.memset(oneb, 1.0)

            # streaming pools for the main loop (declared early so the first
            # block's noise/adj DMAs can be issued ahead of the rest of
            # phase 0)
            spool = ctx.enter_context(tc.tile_pool(name="stream", bufs=3))
            wpool = ctx.enter_context(tc.tile_pool(name="work", bufs=2))
            rpool = ctx.enter_context(tc.tile_pool(name="smalls", bufs=4))

            def issue_dma(ib):
                nz = spool.tile([128, N], bf16, tag="nz", name=f"nz{ib}")
                nc.sync.dma_start(out=nz, in_=wn_d[ib * 128:(ib + 1) * 128, :])
                ad = spool.tile([128, N], bf16, tag="ad", name=f"ad{ib}")
                nc.sync.dma_start(out=ad, in_=adj_d[ib * 128:(ib + 1) * 128, :])
                return nz, ad

            def stage_early(ib, nz):
                if ib in ROUTE_A:
                    # v then g' = -gumbel, both in place in bf16
                    nc.scalar.activation(out=nz, in_=nz, func=AF.Ln, bias=oneb, scale=-1.0)
                    nc.scalar.activation(out=nz, in_=nz, func=AF.Ln, bias=epsb, scale=-1.0)
                    return None
                v = wpool.tile([128, N], f32, tag="v", name=f"v{ib}")
                nc.scalar.activation(out=v, in_=nz, func=AF.Ln, bias=oneb, scale=-1.0)
                r = wpool.tile([128, N], f32, tag="r", name=f"r{ib}")
                nc.vector.reciprocal_approx_fast(out=r, in_=v)   # r = 1/v < 0
                return r

            # ---------------- phase 0 ----------------
            early_q = {}
            with tc.tile_pool(name="p0", bufs=1) as p0:
                xT2 = [p0.tile([128, N], bf16, tag=f"xT{fc}", name=f"xT{fc}") for fc in range(2)]
                xTr2 = [p0.tile([128, RB], bf16, tag=f"xTr{fc}", name=f"xTr{fc}") for fc in range(2)]
                wsdt = [p0.tile([128, 2], bf16, tag=f"wsd{fc}", name=f"wsdt{fc}") for fc in range(2)]
                WWot = [p0.tile([128, OUT_F], bf16, tag=f"WWo{fc}", name=f"WWot{fc}") for fc in range(2)]
                dma_q = {0: issue_dma(0)}
                for fc in range(2):
                    nc.sync.dma_start(out=wsdt[fc], in_=wsd_d[fc * 128:(fc + 1) * 128, :])
                    nc.sync.dma_start(out=xT2[fc], in_=xT_d[fc * 128:(fc + 1) * 128, :])
                dma_q[1] = issue_dma(1)
                for fc in range(2):
                    nc.sync.dma_start(out=xTr2[fc], in_=xTr_d[fc * 128:(fc + 1) * 128, :])
                    nc.sync.dma_start(out=WWot[fc], in_=WWo_d[fc * 128:(fc + 1) * 128, :])

                # ACT can start on block 0 immediately (depends only on nz DMA)
                nz0, ad0 = dma_q.pop(0)
                early_q[0] = (nz0, ad0, stage_early(0, nz0))

                # s_src/s_dst for ALL nodes: ssd[2, n] = wsd^T @ xT
                with tc.tile_pool(name="ps_a", bufs=1, space="PSUM") as ps_a:
                    ssd_ps = [ps_a.tile([2, 512], f32, tag=f"ssd{jc}", name=f"ssd{jc}")
                              for jc in range(8)]
                    for jc in range(8):
                        for fc in range(2):
                            nc.tensor.matmul(ssd_ps[jc], wsdt[fc],
                                             xT2[fc][:, jc * 512:(jc + 1) * 512],
                                             start=(fc == 0), stop=(fc == 1))
                    # rows: 0 = s_dst(all nodes), 1 = s_src(all nodes) [unused]
                    for jc in range(8):
                        sl = slice(jc * 512, (jc + 1) * 512)
                        # raw sd (route A): ACT Copy, psum -> sbuf bf16
                        nc.scalar.copy(sd_rows[0:1, sl], ssd_ps[jc][0:1, :])
                        # exp sd (route B): ACT Exp
                        nc.scalar.activation(out=sd_rows[0:1, N + jc * 512:N + (jc + 1) * 512],
                                             in_=ssd_ps[jc][0:1, :], func=AF.Exp)

                # broadcast the two rows down 128 partitions via DRAM scratch
                sd_dram = nc.dram_tensor("sd_scratch", [1, 2 * N], bf16)
                nc.sync.dma_start(out=sd_dram[:], in_=sd_rows)
                raw_ap = sd_dram[0:1, 0:N]
                exp_ap = sd_dram[0:1, N:2 * N]
                raw_b = bass_mod.AP(tensor=raw_ap.tensor, offset=raw_ap.offset,
                                    ap=[[0, 128]] + list(raw_ap.ap)[1:])
                exp_b = bass_mod.AP(tensor=exp_ap.tensor, offset=exp_ap.offset,
                                    ap=[[0, 128]] + list(exp_ap.ap)[1:])
                nc.gpsimd.dma_start(out=sdb, in_=raw_b)
                nc.gpsimd.dma_start(out=sdbe, in_=exp_b)

                # ss_col + k tiles
                with tc.tile_pool(name="ps_b", bufs=2, space="PSUM") as ps_b:
                    sscol_ps = ps_b.tile([128, NBLK], f32, tag="sscol")
                    for ib in range(NBLK):
                        for fc in range(2):
                            nc.tensor.matmul(sscol_ps[:, ib:ib + 1],
                                             xTr2[fc][:, ib * 128:(ib + 1) * 128],
                                             wsdt[fc][:, 1:2],
                                             start=(fc == 0), stop=(fc == 1))
                    nc.vector.tensor_copy(ss_col, sscol_ps)
                    nc.scalar.activation(out=ess_col, in_=ss_col, func=AF.Exp)

                    for ib in range(NBLK):
                        kps = ps_b.tile([128, OUT_F], f32, tag="kps")
                        for fc in range(2):
                            nc.tensor.matmul(kps,
                                             xTr2[fc][:, ib * 128:(ib + 1) * 128],
                                             WWot[fc],
                                             start=(fc == 0), stop=(fc == 1))
                        nc.vector.tensor_copy(ktil[ib], kps)

            # ---------------- main loop ----------------
            aggpool = ctx.enter_context(tc.tile_pool(name="agg", bufs=1, space="PSUM"))
            aggp = [aggpool.tile([OUT_F, 512], f32, tag=f"agg{j}", name=f"agg{j}")
                    for j in range(8)]

            def stage_late(ib, nz, ad, r):
                rs1 = rpool.tile([128, 1], f32, tag="rs1")
                if ib in ROUTE_A:
                    m = wpool.tile([128, N], bf16, tag="ew", name=f"m{ib}")
                    # m = (sd_j + ss_i) * adj
                    nc.vector.scalar_tensor_tensor(out=m, in0=sdb,
                                                   scalar=ss_col[:, ib:ib + 1],
                                                   in1=ad, op0=ALU.add, op1=ALU.mult)
                    # t = m - g'   (g' in nz)
                    nc.vector.scalar_tensor_tensor(out=m, in0=nz, scalar=-1.0,
                                                   in1=m, op0=ALU.mult, op1=ALU.add)
                    # e1 = exp(t), rs1 = rowsum
                    nc.scalar.activation(out=m, in_=m, func=AF.Exp, accum_out=rs1)
                    e = m
                else:
                    w2 = wpool.tile([128, N], bf16, tag="ew", name=f"w2{ib}")
                    # w2 = esd_j * ess_i - 1
                    nc.vector.tensor_scalar(out=w2, in0=sdbe,
                                            scalar1=ess_col[:, ib:ib + 1],
                                            scalar2=-1.0,
                                            op0=ALU.mult, op1=ALU.add)
                    # p = w2 * adj
                    eng = nc.gpsimd if GP_TT else nc.vector
                    eng.tensor_tensor(out=w2, in0=w2, in1=ad, op=ALU.mult)
                    # e1 = (p + 1) * r   (negative), rs1 = rowsum (negative)
                    nc.vector.scalar_tensor_tensor(out=w2, in0=w2, scalar=1.0,
                                                   in1=r, op0=ALU.add, op1=ALU.mult,
                                                   accum_out=rs1)
                    e = w2

                rs1r = rpool.tile([128, 1], f32, tag="rs1r")
                nc.vector.reciprocal(rs1r, rs1)
                # e2 = exp(e1/rs1), rs2 = rowsum(e2)
                rs2 = rpool.tile([128, 1], f32, tag="rs2")
                nc.scalar.activation(out=e, in_=e, func=AF.Exp, scale=rs1r,
                                     accum_out=rs2)
                rs2r = rpool.tile([128, 1], f32, tag="rs2r")
                nc.vector.reciprocal(rs2r, rs2)
                # k~ = k / rs2
                kt = rpool.tile([128, OUT_F], bf16, tag="kt")
                nc.vector.tensor_scalar(out=kt, in0=ktil[ib], scalar1=rs2r,
                                        scalar2=None, op0=ALU.mult)
                # outT += k~^T @ e2 accumulated in PSUM across blocks
                for ns in range(8):
                    nc.tensor.matmul(aggp[ns], kt,
                                     e[:, ns * 512:(ns + 1) * 512],
                                     start=(ib == 0), stop=(ib == NBLK - 1))

            for ib in range(NBLK):
                if ib + 2 < NBLK:
                    dma_q[ib + 2] = issue_dma(ib + 2)
                if ib + 1 < NBLK:
                    nz1, ad1 = dma_q.pop(ib + 1)
                    early_q[ib + 1] = (nz1, ad1, stage_early(ib + 1, nz1))
                nz, ad, r = early_q.pop(ib)
                stage_late(ib, nz, ad, r)

            # ---------------- epilogue ----------------
            with tc.tile_pool(name="fin", bufs=1) as fpool:
                outT = fpool.tile([OUT_F, N], f32)
                for ns in range(8):
                    nc.scalar.copy(outT[:, ns * 512:(ns + 1) * 512], aggp[ns])
                nc.sync.dma_start(out=outT_d[:], in_=outT)

    # Compile with table lists filtered so Ln and Exp both resolve to the
    # combined 'natural_log_exp_and_others' set (single ACT_TABLE_LOAD).
    import concourse.hw_specs as hw_specs

    orig_fn = bacc.get_activation_tables

    def patched(arch):
        tabs = dict(orig_fn(arch))
        ln, ex = mybir.ActivationFunctionType.Ln, mybir.ActivationFunctionType.Exp
        for name in list(tabs):
            if name != "natural_log_exp_and_others":
                tabs[name] = tabs[name] - {ln, ex}
        return tabs

    bacc.get_activation_tables = patched
    try:
        nc.compile()
    finally:
        bacc.get_activation_tables = orig_fn
    return nc


def _get_module():
    if "nc" not in _cache:
        _cache["nc"] = _build_module()
    return _cache["nc"]


def kernel(x, adj, noise, W, a_src, a_dst, W_out):
    from concourse.bass_utils import run_bass_kernel_spmd

    nc = _get_module()

    bfdt = ml_dtypes.bfloat16
    x = np.asarray(x, dtype=np.float32)
    adj = np.asarray(adj, dtype=np.float32)
    noise = np.asarray(noise, dtype=np.float32)
    W = np.asarray(W, dtype=np.float32)
    a_src = np.asarray(a_src, dtype=np.float32)
    a_dst = np.asarray(a_dst, dtype=np.float32)
    W_out = np.asarray(W_out, dtype=np.float32)

    # fold the per-head score weights: s = (x @ W) @ a_flat / H == x @ (W @ a_flat / H)
    w_src = (W @ a_src.reshape(-1)) / H
    w_dst = (W @ a_dst.reshape(-1)) / H
    # column 0 = dst so the ssd PSUM row read starts at partition 0
    wsd = np.stack([w_dst, w_src], axis=1).astype(bfdt)
    WWo = (W @ W_out).astype(bfdt)
    adj_bf = adj.astype(bfdt)  # exact for 0/1 values
    # w = 1-u encoding keeps full relative precision where it matters (u->1);
    # clamp so Ln(1-w) never sees a zero argument after the bf16 cast.
    wn = np.clip(1.0 - noise, 2.0**-24, 0.99609375).astype(bfdt)

    in_maps = []
    for core in range(N_CORES):
        b, rb = core // 2, core % 2
        rows = slice(rb * RB, (rb + 1) * RB)
        xTb = np.ascontiguousarray(x[b].T).astype(bfdt)  # [IN_F, N]
        in_maps.append({
            "xT": xTb,
            "xTr": np.ascontiguousarray(xTb[:, rows]),
            "adj_s": np.ascontiguousarray(adj_bf[rows, :]),
            "wn_s": np.ascontiguousarray(wn[b, rows, :]),
            "wsd": wsd,
            "WWo": WWo,
        })

    res = run_bass_kernel_spmd(nc, in_maps, list(range(N_CORES)))
    kernel._last_results = res

    out = np.empty((B, N, OUT_F), dtype=np.float32)
    for b in range(B):
        acc = res.results[2 * b]["outT"].astype(np.float32) + \
            res.results[2 * b + 1]["outT"].astype(np.float32)
        out[b] = acc.T
    return out


# revision 14
# speedup vs baseline: 1.2856x; 1.0342x over previous
"""GSAT graph-attention kernel for 8 Trainium2 NeuronCores.

Math (per batch b):
  h = x @ W                                     [N, 512]
  ss[i] = h[i] . a_src_flat / H ; sd[j] = h[j] . a_dst_flat / H
  t[i,j] = (ss[i] + sd[j]) * adj[i,j] + gumbel(noise[b,i,j])
  A1 = softmax_j(t) ; A2 = softmax_j(A1)
  out[b,n] = sum_i A2[i,n] * (h[i] @ W_out)

Sharding: 8 cores = (batch b in 0..3) x (row-half rb in 0..1).  Rows i are
sharded; both softmaxes are along j (within-row), so each core computes its
2048 rows completely and produces a partial output outT summed over its
rows; host adds the two row-half partials per batch.

Engine budget note: the elementwise chain is 3 transcendentals + arith per
element.  ACT runs ~1 elem/cycle dtype-independent; DVE runs 2x on all-bf16
tensor ops.  Two per-block schedules are mixed to balance ACT vs DVE:

  route A (ACT-heavy, 4 ACT passes):
     v  = Ln(1 - w)            [ACT, bf16]        (w = 1-u host-encoded)
     g' = Ln(eps - v)          [ACT, bf16]        (= -gumbel)
     m  = (sd_j + ss_i)*adj    [DVE stt, bf16 2x]
     t  = m - g'               [DVE stt, bf16 2x]
     e1 = Exp(t)        accum rs1   [ACT]
     e2 = Exp(e1/rs1)   accum rs2   [ACT]

  route B (DVE-heavy, 2 ACT passes), using
  exp(scores+gumbel) = (1 + adj*(exp(ss_i)exp(sd_j) - 1)) * 1/(-ln(u)):
     v  = Ln(1 - w)            [ACT, f32 out]
     r  = 1/v                  [DVE reciprocal_approx_fast]  (NEGATIVE)
     w2 = esd_j*ess_i - 1      [DVE ts, bf16 4x]
     p  = w2 * adj             [GPSIMD tt (offload) or DVE tt bf16 2x]
     e1 = (p + 1) * r   accum rs1   [DVE stt]    (e1, rs1 both negative;
     e2 = Exp(e1/rs1)   accum rs2   [ACT]         the sign cancels in e1/rs1)

W_out is folded before aggregation ((A^T H)Wo = A^T(H Wo)); k = x @ (W Wo)
with W Wo folded on the host.  1/rs2 is folded into k.  Both Ln and Exp live
in the 'natural_log_exp_and_others' ACT table set; compile-time table lists
are filtered so the fixpoint pass picks that set (1 table load instead of the
per-block ping-pong between natural_log and exp_and_others).
"""

import os
import sys

for _p in ("/opt/trn_rl_repo",):
    if _p not in sys.path and os.path.isdir(_p):
        sys.path.insert(0, _p)

os.environ.setdefault("MYCRO_LOCAL_CACHE", "1")

import numpy as np
import ml_dtypes

B, N, IN_F, H, OUT_F = 4, 4096, 256, 8, 64
D = H * OUT_F          # 512
RB = N // 2            # 2048 rows per core
NBLK = RB // 128       # 16 row blocks per core
EPS = 1e-9
N_CORES = 8

# Tuning knobs (env overrides are for local experiments only; defaults baked)
ROUTE_A = set(int(x) for x in os.environ.get("KRN_ROUTE_A", "5,9,13").split(",") if x != "")
GP_TT = os.environ.get("KRN_GP_TT", "1") == "1"   # B-route mask-mult on GPSIMD

_cache = {}


def _build_module():
    import contextlib

    import concourse.bacc as bacc
    import concourse.bass as bass_mod
    import concourse.tile as tile
    from concourse import mybir

    f32 = mybir.dt.float32
    bf16 = mybir.dt.bfloat16
    AF = mybir.ActivationFunctionType
    ALU = mybir.AluOpType

    nc = bacc.Bacc("TRN2", target_bir_lowering=False)

    xT_d = nc.declare_dram_parameter("xT", [IN_F, N], bf16, isOutput=False)
    xTr_d = nc.declare_dram_parameter("xTr", [IN_F, RB], bf16, isOutput=False)
    adj_d = nc.declare_dram_parameter("adj_s", [RB, N], bf16, isOutput=False)
    wn_d = nc.declare_dram_parameter("wn_s", [RB, N], bf16, isOutput=False)
    wsd_d = nc.declare_dram_parameter("wsd", [IN_F, 2], bf16, isOutput=False)
    WWo_d = nc.declare_dram_parameter("WWo", [IN_F, OUT_F], bf16, isOutput=False)
    outT_d = nc.declare_dram_parameter("outT", [OUT_F, N], f32, isOutput=True)

    with tile.TileContext(nc) as tc:
        with contextlib.ExitStack() as ctx:
            pers = ctx.enter_context(tc.tile_pool(name="pers", bufs=1))
            sdb = pers.tile([128, N], bf16)       # raw s_dst broadcast (route A)
            sdbe = pers.tile([128, N], bf16)      # exp(s_dst) broadcast (route B)
            ss_col = pers.tile([128, NBLK], f32)  # ss_col[p, b] = s_src[b*128+p]
            ess_col = pers.tile([128, NBLK], f32)  # exp(ss_col)
            ktil = [pers.tile([128, OUT_F], f32, tag=f"k{ib}", name=f"k{ib}")
                    for ib in range(NBLK)]
            sd_rows = pers.tile([1, 2 * N], bf16)  # [raw sd | exp sd] rows
            epsb = pers.tile([128, 1], f32)
            nc.vector.memset(epsb, EPS)
            oneb = pers.tile([128, 1], f32)
            nc.vector.memset(oneb, 1.0)

            # streaming pools for the main loop (declared early so the first
            # block's noise/adj DMAs can be issued ahead of the rest of
            # phase 0)
            spool = ctx.enter_context(tc.tile_pool(name="stream", bufs=3))
            wpool = ctx.enter_context(tc.tile_pool(name="work", bufs=2))
            mpool = ctx.enter_context(tc.tile_pool(name="mwork", bufs=1))
            rpool = ctx.enter_context(tc.tile_pool(name="smalls", bufs=4))

            def issue_dma(ib):
                nz = spool.tile([128, N], bf16, tag="nz", name=f"nz{ib}")
                nc.sync.dma_start(out=nz, in_=wn_d[ib * 128:(ib + 1) * 128, :])
                ad = spool.tile([128, N], bf16, tag="ad", name=f"ad{ib}")
                nc.sync.dma_start(out=ad, in_=adj_d[ib * 128:(ib + 1) * 128, :])
                return nz, ad

            def stage_early(ib, nz):
                if ib in ROUTE_A:
                    # v then g' = -gumbel, both in place in bf16
                    nc.scalar.activation(out=nz, in_=nz, func=AF.Ln, bias=oneb, scale=-1.0)
                    nc.scalar.activation(out=nz, in_=nz, func=AF.Ln, bias=epsb, scale=-1.0)
                    return None
                v = wpool.tile([128, N], f32, tag="v", name=f"v{ib}")
                nc.scalar.activation(out=v, in_=nz, func=AF.Ln, bias=oneb, scale=-1.0)
                r = wpool.tile([128, N], f32, tag="r", name=f"r{ib}")
                nc.vector.reciprocal_approx_fast(out=r, in_=v)   # r = 1/v < 0
                return r

            # stage_mid needs phase-0 products (sdbe, ess_col); first issued
            # after phase 0 closes, then one block ahead inside the loop.
            def stage_mid(ib, ad):
                if ib in ROUTE_A:
                    return None
                # w2 = esd_j * ess_i - 1  (ts hits 4x bf16; stt would be 1x)
                w2 = wpool.tile([128, N], bf16, tag="ew", name=f"w2{ib}")
                nc.vector.tensor_scalar(out=w2, in0=sdbe,
                                        scalar1=ess_col[:, ib:ib + 1],
                                        scalar2=-1.0,
                                        op0=ALU.mult, op1=ALU.add)
                # p = w2 * adj on GPSIMD (or DVE) one block ahead of use
                eng = nc.gpsimd if GP_TT else nc.vector
                eng.tensor_tensor(out=w2, in0=w2, in1=ad, op=ALU.mult)
                return w2

            # ---------------- phase 0 ----------------
            early_q = {}
            with tc.tile_pool(name="p0", bufs=1) as p0:
                xT2 = [p0.tile([128, N], bf16, tag=f"xT{fc}", name=f"xT{fc}") for fc in range(2)]
                xTr2 = [p0.tile([128, RB], bf16, tag=f"xTr{fc}", name=f"xTr{fc}") for fc in range(2)]
                wsdt = [p0.tile([128, 2], bf16, tag=f"wsd{fc}", name=f"wsdt{fc}") for fc in range(2)]
                WWot = [p0.tile([128, OUT_F], bf16, tag=f"WWo{fc}", name=f"WWot{fc}") for fc in range(2)]
                dma_q = {0: issue_dma(0)}
                for fc in range(2):
                    nc.sync.dma_start(out=wsdt[fc], in_=wsd_d[fc * 128:(fc + 1) * 128, :])
                    nc.sync.dma_start(out=xT2[fc], in_=xT_d[fc * 128:(fc + 1) * 128, :])
                dma_q[1] = issue_dma(1)
                for fc in range(2):
                    nc.sync.dma_start(out=xTr2[fc], in_=xTr_d[fc * 128:(fc + 1) * 128, :])
                    nc.sync.dma_start(out=WWot[fc], in_=WWo_d[fc * 128:(fc + 1) * 128, :])

                # ACT can start on block 0 immediately (depends only on nz DMA)
                nz0, ad0 = dma_q.pop(0)
                early_q[0] = (nz0, ad0, stage_early(0, nz0))

                # s_src/s_dst for ALL nodes: ssd[2, n] = wsd^T @ xT
                with tc.tile_pool(name="ps_a", bufs=1, space="PSUM") as ps_a:
                    ssd_ps = [ps_a.tile([2, 512], f32, tag=f"ssd{jc}", name=f"ssd{jc}")
                              for jc in range(8)]
                    for jc in range(8):
                        for fc in range(2):
                            nc.tensor.matmul(ssd_ps[jc], wsdt[fc],
                                             xT2[fc][:, jc * 512:(jc + 1) * 512],
                                             start=(fc == 0), stop=(fc == 1))
                    # rows: 0 = s_dst(all nodes), 1 = s_src(all nodes) [unused]
                    for jc in range(8):
                        sl = slice(jc * 512, (jc + 1) * 512)
                        # raw sd (route A): ACT Copy, psum -> sbuf bf16
                        nc.scalar.copy(sd_rows[0:1, sl], ssd_ps[jc][0:1, :])
                        # exp sd (route B): ACT Exp
                        nc.scalar.activation(out=sd_rows[0:1, N + jc * 512:N + (jc + 1) * 512],
                                             in_=ssd_ps[jc][0:1, :], func=AF.Exp)

                # broadcast the two rows down 128 partitions via DRAM scratch
                sd_dram = nc.dram_tensor("sd_scratch", [1, 2 * N], bf16)
                nc.sync.dma_start(out=sd_dram[:], in_=sd_rows)
                raw_ap = sd_dram[0:1, 0:N]
                exp_ap = sd_dram[0:1, N:2 * N]
                raw_b = bass_mod.AP(tensor=raw_ap.tensor, offset=raw_ap.offset,
                                    ap=[[0, 128]] + list(raw_ap.ap)[1:])
                exp_b = bass_mod.AP(tensor=exp_ap.tensor, offset=exp_ap.offset,
                                    ap=[[0, 128]] + list(exp_ap.ap)[1:])
                nc.gpsimd.dma_start(out=sdb, in_=raw_b)
                nc.gpsimd.dma_start(out=sdbe, in_=exp_b)

                # ss_col + k tiles
                with tc.tile_pool(name="ps_b", bufs=2, space="PSUM") as ps_b:
                    sscol_ps = ps_b.tile([128, NBLK], f32, tag="sscol")
                    for ib in range(NBLK):
                        for fc in range(2):
                            nc.tensor.matmul(sscol_ps[:, ib:ib + 1],
                                             xTr2[fc][:, ib * 128:(ib + 1) * 128],
                                             wsdt[fc][:, 1:2],
                                             start=(fc == 0), stop=(fc == 1))
                    nc.vector.tensor_copy(ss_col, sscol_ps)
                    nc.scalar.activation(out=ess_col, in_=ss_col, func=AF.Exp)

                    for ib in range(NBLK):
                        kps = ps_b.tile([128, OUT_F], f32, tag="kps")
                        for fc in range(2):
                            nc.tensor.matmul(kps,
                                             xTr2[fc][:, ib * 128:(ib + 1) * 128],
                                             WWot[fc],
                                             start=(fc == 0), stop=(fc == 1))
                        nc.vector.tensor_copy(ktil[ib], kps)

            # ---------------- main loop ----------------
            aggpool = ctx.enter_context(tc.tile_pool(name="agg", bufs=1, space="PSUM"))
            aggp = [aggpool.tile([OUT_F, 512], f32, tag=f"agg{j}", name=f"agg{j}")
                    for j in range(8)]

            def stage_late(ib, nz, ad, r, w2):
                rs1 = rpool.tile([128, 1], f32, tag="rs1")
                if ib in ROUTE_A:
                    m = mpool.tile([128, N], bf16, tag="m", name=f"m{ib}")
                    # m = sd_j + ss_i  (ts 4x)
                    nc.vector.tensor_scalar(out=m, in0=sdb,
                                            scalar1=ss_col[:, ib:ib + 1],
                                            scalar2=None, op0=ALU.add)
                    # m *= adj  (tt 2x)
                    nc.vector.tensor_tensor(out=m, in0=m, in1=ad, op=ALU.mult)
                    # t = m - g'   (g' in nz; tt 2x)
                    nc.vector.tensor_tensor(out=m, in0=m, in1=nz, op=ALU.subtract)
                    # e1 = exp(t), rs1 = rowsum
                    nc.scalar.activation(out=m, in_=m, func=AF.Exp, accum_out=rs1)
                    e = m
                else:
                    # e1 = (p + 1) * r   (negative), rs1 = rowsum (negative)
                    nc.vector.scalar_tensor_tensor(out=w2, in0=w2, scalar=1.0,
                                                   in1=r, op0=ALU.add, op1=ALU.mult,
                                                   accum_out=rs1)
                    e = w2

                rs1r = rpool.tile([128, 1], f32, tag="rs1r")
                nc.vector.reciprocal(rs1r, rs1)
                # e2 = exp(e1/rs1), rs2 = rowsum(e2)
                rs2 = rpool.tile([128, 1], f32, tag="rs2")
                nc.scalar.activation(out=e, in_=e, func=AF.Exp, scale=rs1r,
                                     accum_out=rs2)
                rs2r = rpool.tile([128, 1], f32, tag="rs2r")
                nc.vector.reciprocal(rs2r, rs2)
                # k~ = k / rs2
                kt = rpool.tile([128, OUT_F], bf16, tag="kt")
                nc.vector.tensor_scalar(out=kt, in0=ktil[ib], scalar1=rs2r,
                                        scalar2=None, op0=ALU.mult)
                # outT += k~^T @ e2 accumulated in PSUM across blocks
                for ns in range(8):
                    nc.tensor.matmul(aggp[ns], kt,
                                     e[:, ns * 512:(ns + 1) * 512],
                                     start=(ib == 0), stop=(ib == NBLK - 1))

            mid_q = {0: stage_mid(0, early_q[0][1])}
            for ib in range(NBLK):
                if ib + 2 < NBLK:
                    dma_q[ib + 2] = issue_dma(ib + 2)
                if ib + 1 < NBLK:
                    nz1, ad1 = dma_q.pop(ib + 1)
                    early_q[ib + 1] = (nz1, ad1, stage_early(ib + 1, nz1))
                    mid_q[ib + 1] = stage_mid(ib + 1, ad1)
                nz, ad, r = early_q.pop(ib)
                stage_late(ib, nz, ad, r, mid_q.pop(ib))

            # ---------------- epilogue ----------------
            with tc.tile_pool(name="fin", bufs=1) as fpool:
                outT = fpool.tile([OUT_F, N], f32)
                for ns in range(8):
                    nc.scalar.copy(outT[:, ns * 512:(ns + 1) * 512], aggp[ns])
                nc.sync.dma_start(out=outT_d[:], in_=outT)

    # Compile with table lists filtered so Ln and Exp both resolve to the
    # combined 'natural_log_exp_and_others' set (single ACT_TABLE_LOAD).
    import concourse.hw_specs as hw_specs

    orig_fn = bacc.get_activation_tables

    def patched(arch):
        tabs = dict(orig_fn(arch))
        ln, ex = mybir.ActivationFunctionType.Ln, mybir.ActivationFunctionType.Exp
        for name in list(tabs):
            if name != "natural_log_exp_and_others":
                tabs[name] = tabs[name] - {ln, ex}
        return tabs

    bacc.get_activation_tables = patched
    try:
        nc.compile()
    finally:
        bacc.get_activation_tables = orig_fn
    return nc


def _get_module():
    if "nc" not in _cache:
        _cache["nc"] = _build_module()
    return _cache["nc"]


def kernel(x, adj, noise, W, a_src, a_dst, W_out):
    from concourse.bass_utils import run_bass_kernel_spmd

    nc = _get_module()

    bfdt = ml_dtypes.bfloat16
    x = np.asarray(x, dtype=np.float32)
    adj = np.asarray(adj, dtype=np.float32)
    noise = np.asarray(noise, dtype=np.float32)
    W = np.asarray(W, dtype=np.float32)
    a_src = np.asarray(a_src, dtype=np.float32)
    a_dst = np.asarray(a_dst, dtype=np.float32)
    W_out = np.asarray(W_out, dtype=np.float32)

    # fold the per-head score weights: s = (x @ W) @ a_flat / H == x @ (W @ a_flat / H)
    w_src = (W @ a_src.reshape(-1)) / H
    w_dst = (W @ a_dst.reshape(-1)) / H
    # column 0 = dst so the ssd PSUM row read starts at partition 0
    wsd = np.stack([w_dst, w_src], axis=1).astype(bfdt)
    WWo = (W @ W_out).astype(bfdt)
    adj_bf = adj.astype(bfdt)  # exact for 0/1 values
    # w = 1-u encoding keeps full relative precision where it matters (u->1);
    # clamp so Ln(1-w) never sees a zero argument after the bf16 cast.
    wn = np.clip(1.0 - noise, 2.0**-24, 0.99609375).astype(bfdt)

    in_maps = []
    for core in range(N_CORES):
        b, rb = core // 2, core % 2
        rows = slice(rb * RB, (rb + 1) * RB)
        xTb = np.ascontiguousarray(x[b].T).astype(bfdt)  # [IN_F, N]
        in_maps.append({
            "xT": xTb,
            "xTr": np.ascontiguousarray(xTb[:, rows]),
            "adj_s": np.ascontiguousarray(adj_bf[rows, :]),
            "wn_s": np.ascontiguousarray(wn[b, rows, :]),
            "wsd": wsd,
            "WWo": WWo,
        })

    res = run_bass_kernel_spmd(nc, in_maps, list(range(N_CORES)))
    kernel._last_results = res

    out = np.empty((B, N, OUT_F), dtype=np.float32)
    for b in range(B):
        acc = res.results[2 * b]["outT"].astype(np.float32) + \
            res.results[2 * b + 1]["outT"].astype(np.float32)
        out[b] = acc.T
    return out


# revision 16
# speedup vs baseline: 1.4552x; 1.1320x over previous
"""GSAT graph-attention kernel for 8 Trainium2 NeuronCores.

Math (per batch b):
  h = x @ W                                     [N, 512]
  ss[i] = h[i] . a_src_flat / H ; sd[j] = h[j] . a_dst_flat / H
  t[i,j] = (ss[i] + sd[j]) * adj[i,j] + gumbel(noise[b,i,j])
  A1 = softmax_j(t) ; A2 = softmax_j(A1)
  out[b,n] = sum_i A2[i,n] * (h[i] @ W_out)

Sharding: 8 cores = (batch b in 0..3) x (row-half rb in 0..1).  Rows i are
sharded; both softmaxes are along j (within-row), so each core computes its
2048 rows completely and produces a partial output outT summed over its
rows; host adds the two row-half partials per batch.

Engine budget note: the elementwise chain is 3 transcendentals + arith per
element.  ACT runs ~1 elem/cycle dtype-independent; DVE runs 2x on all-bf16
tensor ops.  Two per-block schedules are mixed to balance ACT vs DVE:

  route A (ACT-heavy, 4 ACT passes):
     v  = Ln(1 - w)            [ACT, bf16]        (w = 1-u host-encoded)
     g' = Ln(eps - v)          [ACT, bf16]        (= -gumbel)
     m  = (sd_j + ss_i)*adj    [DVE stt, bf16 2x]
     t  = m - g'               [DVE stt, bf16 2x]
     e1 = Exp(t)        accum rs1   [ACT]
     e2 = Exp(e1/rs1)   accum rs2   [ACT]

  route B (DVE-heavy, 2 ACT passes), using
  exp(scores+gumbel) = (1 + adj*(exp(ss_i)exp(sd_j) - 1)) * 1/(-ln(u)):
     v  = Ln(1 - w)            [ACT, f32 out]
     r  = 1/v                  [DVE reciprocal_approx_fast]  (NEGATIVE)
     w2 = esd_j*ess_i - 1      [DVE ts, bf16 4x]
     p  = w2 * adj             [GPSIMD tt (offload) or DVE tt bf16 2x]
     e1 = (p + 1) * r   accum rs1   [DVE stt]    (e1, rs1 both negative;
     e2 = Exp(e1/rs1)   accum rs2   [ACT]         the sign cancels in e1/rs1)

W_out is folded before aggregation ((A^T H)Wo = A^T(H Wo)); k = x @ (W Wo)
with W Wo folded on the host.  1/rs2 is folded into k.  Both Ln and Exp live
in the 'natural_log_exp_and_others' ACT table set; compile-time table lists
are filtered so the fixpoint pass picks that set (1 table load instead of the
per-block ping-pong between natural_log and exp_and_others).
"""

import os
import sys

for _p in ("/opt/trn_rl_repo",):
    if _p not in sys.path and os.path.isdir(_p):
        sys.path.insert(0, _p)

os.environ.setdefault("MYCRO_LOCAL_CACHE", "1")

import numpy as np
import ml_dtypes

B, N, IN_F, H, OUT_F = 4, 4096, 256, 8, 64
D = H * OUT_F          # 512
RB = N // 2            # 2048 rows per core
NBLK = RB // 128       # 16 row blocks per core
EPS = 1e-9
N_CORES = 8

# Tuning knobs (env overrides are for local experiments only; defaults baked)
ROUTE_A = set(int(x) for x in os.environ.get("KRN_ROUTE_A", "5,9,13").split(",") if x != "")
GP_TT = os.environ.get("KRN_GP_TT", "1") == "1"   # B-route mask-mult on GPSIMD

_cache = {}


def _build_module():
    import contextlib

    import concourse.bacc as bacc
    import concourse.bass as bass_mod
    import concourse.tile as tile
    from concourse import mybir

    f32 = mybir.dt.float32
    bf16 = mybir.dt.bfloat16
    AF = mybir.ActivationFunctionType
    ALU = mybir.AluOpType

    nc = bacc.Bacc("TRN2", target_bir_lowering=False)

    xT_d = nc.declare_dram_parameter("xT", [IN_F, N], bf16, isOutput=False)
    xTr_d = nc.declare_dram_parameter("xTr", [IN_F, RB], bf16, isOutput=False)
    adj_d = nc.declare_dram_parameter("adj_s", [RB, N], bf16, isOutput=False)
    wn_d = nc.declare_dram_parameter("wn_s", [RB, N], bf16, isOutput=False)
    wsd_d = nc.declare_dram_parameter("wsd", [IN_F, 2], bf16, isOutput=False)
    WWo_d = nc.declare_dram_parameter("WWo", [IN_F, OUT_F], bf16, isOutput=False)
    outT_d = nc.declare_dram_parameter("outT", [OUT_F, N], f32, isOutput=True)

    with tile.TileContext(nc) as tc:
        with contextlib.ExitStack() as ctx:
            pers = ctx.enter_context(tc.tile_pool(name="pers", bufs=1))
            sdb = pers.tile([128, N], bf16)       # raw s_dst broadcast (route A)
            sdbe = pers.tile([128, N], bf16)      # exp(s_dst) broadcast (route B)
            ss_col = pers.tile([128, NBLK], f32)  # ss_col[p, b] = s_src[b*128+p]
            ess_col = pers.tile([128, NBLK], f32)  # exp(ss_col)
            ktil = [pers.tile([128, OUT_F], f32, tag=f"k{ib}", name=f"k{ib}")
                    for ib in range(NBLK)]
            epsb = pers.tile([128, 1], f32)
            nc.vector.memset(epsb, EPS)
            oneb = pers.tile([128, 1], f32)
            nc.vector.memset(oneb, 1.0)

            # streaming pools for the main loop (declared early so the first
            # block's noise/adj DMAs can be issued ahead of the rest of
            # phase 0)
            nzpool = ctx.enter_context(tc.tile_pool(name="nzs", bufs=4))
            adpool = ctx.enter_context(tc.tile_pool(name="ads", bufs=4))
            wpool = ctx.enter_context(tc.tile_pool(name="work", bufs=2))
            vpool = ctx.enter_context(tc.tile_pool(name="vwork", bufs=1))
            epool = ctx.enter_context(tc.tile_pool(name="ework", bufs=3))
            mpool = ctx.enter_context(tc.tile_pool(name="mwork", bufs=1))
            rpool = ctx.enter_context(tc.tile_pool(name="smalls", bufs=4))

            def issue_dma(ib):
                nz = nzpool.tile([128, N], bf16, tag="nz", name=f"nz{ib}")
                nc.sync.dma_start(out=nz, in_=wn_d[ib * 128:(ib + 1) * 128, :])
                ad = adpool.tile([128, N], bf16, tag="ad", name=f"ad{ib}")
                nc.sync.dma_start(out=ad, in_=adj_d[ib * 128:(ib + 1) * 128, :])
                return nz, ad

            def stage_early(ib, nz):
                if ib in ROUTE_A:
                    # v then g' = -gumbel, both in place in bf16
                    nc.scalar.activation(out=nz, in_=nz, func=AF.Ln, bias=oneb, scale=-1.0)
                    nc.scalar.activation(out=nz, in_=nz, func=AF.Ln, bias=epsb, scale=-1.0)
                    return None
                v = vpool.tile([128, N], f32, tag="v", name=f"v{ib}")
                nc.scalar.activation(out=v, in_=nz, func=AF.Ln, bias=oneb, scale=-1.0)
                r = wpool.tile([128, N], f32, tag="r", name=f"r{ib}")
                nc.vector.reciprocal_approx_fast(out=r, in_=v)   # r = 1/v < 0
                return r

            # stage_mid needs phase-0 products (sdbe, ess_col); first issued
            # after phase 0 closes, then one block ahead inside the loop.
            def stage_mid(ib, ad):
                if ib in ROUTE_A:
                    return None
                # w2 = esd_j * ess_i - 1  (ts hits 4x bf16; stt would be 1x)
                w2 = epool.tile([128, N], bf16, tag="ew", name=f"w2{ib}")
                nc.vector.tensor_scalar(out=w2, in0=sdbe,
                                        scalar1=ess_col[:, ib:ib + 1],
                                        scalar2=-1.0,
                                        op0=ALU.mult, op1=ALU.add)
                # p = w2 * adj on GPSIMD (or DVE) one block ahead of use
                eng = nc.gpsimd if GP_TT else nc.vector
                eng.tensor_tensor(out=w2, in0=w2, in1=ad, op=ALU.mult)
                return w2

            # ---------------- phase 0 ----------------
            early_q = {}
            with tc.tile_pool(name="p0", bufs=1) as p0:
                xT2 = [p0.tile([128, N], bf16, tag=f"xT{fc}", name=f"xT{fc}") for fc in range(2)]
                xTr2 = [p0.tile([128, RB], bf16, tag=f"xTr{fc}", name=f"xTr{fc}") for fc in range(2)]
                wsdt = [p0.tile([128, 2], bf16, tag=f"wsd{fc}", name=f"wsdt{fc}") for fc in range(2)]
                sd_rows = p0.tile([1, 2 * N], bf16)  # [raw sd | exp sd] rows
                WWot = [p0.tile([128, OUT_F], bf16, tag=f"WWo{fc}", name=f"WWot{fc}") for fc in range(2)]
                dma_q = {0: issue_dma(0)}
                for fc in range(2):
                    nc.sync.dma_start(out=wsdt[fc], in_=wsd_d[fc * 128:(fc + 1) * 128, :])
                    nc.sync.dma_start(out=xT2[fc], in_=xT_d[fc * 128:(fc + 1) * 128, :])
                dma_q[1] = issue_dma(1)
                for fc in range(2):
                    nc.sync.dma_start(out=xTr2[fc], in_=xTr_d[fc * 128:(fc + 1) * 128, :])
                    nc.sync.dma_start(out=WWot[fc], in_=WWo_d[fc * 128:(fc + 1) * 128, :])

                dma_q[2] = issue_dma(2)
                # ACT can start on block 0 immediately (depends only on nz DMA)
                nz0, ad0 = dma_q.pop(0)
                early_q[0] = (nz0, ad0, stage_early(0, nz0))

                # s_src/s_dst for ALL nodes: ssd[2, n] = wsd^T @ xT
                with tc.tile_pool(name="ps_a", bufs=1, space="PSUM") as ps_a:
                    ssd_ps = [ps_a.tile([2, 512], f32, tag=f"ssd{jc}", name=f"ssd{jc}")
                              for jc in range(8)]
                    for jc in range(8):
                        for fc in range(2):
                            nc.tensor.matmul(ssd_ps[jc], wsdt[fc],
                                             xT2[fc][:, jc * 512:(jc + 1) * 512],
                                             start=(fc == 0), stop=(fc == 1))
                    # rows: 0 = s_dst(all nodes), 1 = s_src(all nodes) [unused]
                    for jc in range(8):
                        sl = slice(jc * 512, (jc + 1) * 512)
                        # raw sd (route A): ACT Copy, psum -> sbuf bf16
                        nc.scalar.copy(sd_rows[0:1, sl], ssd_ps[jc][0:1, :])
                        # exp sd (route B): ACT Exp
                        nc.scalar.activation(out=sd_rows[0:1, N + jc * 512:N + (jc + 1) * 512],
                                             in_=ssd_ps[jc][0:1, :], func=AF.Exp)

                # broadcast the two rows down 128 partitions via DRAM scratch
                sd_dram = nc.dram_tensor("sd_scratch", [1, 2 * N], bf16)
                nc.sync.dma_start(out=sd_dram[:], in_=sd_rows)
                raw_ap = sd_dram[0:1, 0:N]
                exp_ap = sd_dram[0:1, N:2 * N]
                raw_b = bass_mod.AP(tensor=raw_ap.tensor, offset=raw_ap.offset,
                                    ap=[[0, 128]] + list(raw_ap.ap)[1:])
                exp_b = bass_mod.AP(tensor=exp_ap.tensor, offset=exp_ap.offset,
                                    ap=[[0, 128]] + list(exp_ap.ap)[1:])
                nc.gpsimd.dma_start(out=sdb, in_=raw_b)
                nc.gpsimd.dma_start(out=sdbe, in_=exp_b)

                # ss_col + k tiles
                with tc.tile_pool(name="ps_b", bufs=2, space="PSUM") as ps_b:
                    sscol_ps = ps_b.tile([128, NBLK], f32, tag="sscol")
                    for ib in range(NBLK):
                        for fc in range(2):
                            nc.tensor.matmul(sscol_ps[:, ib:ib + 1],
                                             xTr2[fc][:, ib * 128:(ib + 1) * 128],
                                             wsdt[fc][:, 1:2],
                                             start=(fc == 0), stop=(fc == 1))
                    nc.vector.tensor_copy(ss_col, sscol_ps)
                    nc.scalar.activation(out=ess_col, in_=ss_col, func=AF.Exp)

                    for ib in range(NBLK):
                        kps = ps_b.tile([128, OUT_F], f32, tag="kps")
                        for fc in range(2):
                            nc.tensor.matmul(kps,
                                             xTr2[fc][:, ib * 128:(ib + 1) * 128],
                                             WWot[fc],
                                             start=(fc == 0), stop=(fc == 1))
                        nc.vector.tensor_copy(ktil[ib], kps)

            # ---------------- main loop ----------------
            aggpool = ctx.enter_context(tc.tile_pool(name="agg", bufs=1, space="PSUM"))
            aggp = [aggpool.tile([OUT_F, 512], f32, tag=f"agg{j}", name=f"agg{j}")
                    for j in range(8)]

            def stage_late(ib, nz, ad, r, w2):
                rs1 = rpool.tile([128, 1], f32, tag="rs1")
                if ib in ROUTE_A:
                    m = mpool.tile([128, N], bf16, tag="m", name=f"m{ib}")
                    # m = sd_j*1 + ss_i  (MULTIPLY,ADD ts form hits 4x;
                    # single-op ADD,BYPASS measured 6x slower)
                    nc.vector.tensor_scalar(out=m, in0=sdb,
                                            scalar1=1.0,
                                            scalar2=ss_col[:, ib:ib + 1],
                                            op0=ALU.mult, op1=ALU.add)
                    # m *= adj  (tt 2x)
                    nc.vector.tensor_tensor(out=m, in0=m, in1=ad, op=ALU.mult)
                    # t = m - g'   (g' in nz; tt 2x)
                    nc.vector.tensor_tensor(out=m, in0=m, in1=nz, op=ALU.subtract)
                    # e1 = exp(t), rs1 = rowsum
                    nc.scalar.activation(out=m, in_=m, func=AF.Exp, accum_out=rs1)
                    e = m
                else:
                    # e1 = (p + 1) * r   (negative), rs1 = rowsum (negative)
                    nc.vector.scalar_tensor_tensor(out=w2, in0=w2, scalar=1.0,
                                                   in1=r, op0=ALU.add, op1=ALU.mult,
                                                   accum_out=rs1)
                    e = w2

                rs1r = rpool.tile([128, 1], f32, tag="rs1r")
                nc.vector.reciprocal(rs1r, rs1)
                # e2 = exp(e1/rs1), rs2 = rowsum(e2)
                rs2 = rpool.tile([128, 1], f32, tag="rs2")
                nc.scalar.activation(out=e, in_=e, func=AF.Exp, scale=rs1r,
                                     accum_out=rs2)
                rs2r = rpool.tile([128, 1], f32, tag="rs2r")
                nc.vector.reciprocal(rs2r, rs2)
                # k~ = k / rs2
                kt = rpool.tile([128, OUT_F], bf16, tag="kt")
                nc.vector.tensor_scalar(out=kt, in0=ktil[ib], scalar1=rs2r,
                                        scalar2=None, op0=ALU.mult)
                # outT += k~^T @ e2 accumulated in PSUM across blocks
                for ns in range(8):
                    nc.tensor.matmul(aggp[ns], kt,
                                     e[:, ns * 512:(ns + 1) * 512],
                                     start=(ib == 0), stop=(ib == NBLK - 1))

            mid_q = {0: stage_mid(0, early_q[0][1]),
                     1: stage_mid(1, dma_q[1][1])}
            for ib in range(NBLK):
                if ib + 3 < NBLK:
                    dma_q[ib + 3] = issue_dma(ib + 3)
                if ib + 1 < NBLK:
                    nz1, ad1 = dma_q.pop(ib + 1)
                    early_q[ib + 1] = (nz1, ad1, stage_early(ib + 1, nz1))
                if ib + 2 < NBLK:
                    mid_q[ib + 2] = stage_mid(ib + 2, dma_q[ib + 2][1])
                nz, ad, r = early_q.pop(ib)
                stage_late(ib, nz, ad, r, mid_q.pop(ib))

            # ---------------- epilogue ----------------
            with tc.tile_pool(name="fin", bufs=1) as fpool:
                outT = fpool.tile([OUT_F, N], f32)
                for ns in range(8):
                    nc.scalar.copy(outT[:, ns * 512:(ns + 1) * 512], aggp[ns])
                nc.sync.dma_start(out=outT_d[:], in_=outT)

    # Compile with table lists filtered so Ln and Exp both resolve to the
    # combined 'natural_log_exp_and_others' set (single ACT_TABLE_LOAD).
    import concourse.hw_specs as hw_specs

    orig_fn = bacc.get_activation_tables

    def patched(arch):
        tabs = dict(orig_fn(arch))
        ln, ex = mybir.ActivationFunctionType.Ln, mybir.ActivationFunctionType.Exp
        for name in list(tabs):
            if name != "natural_log_exp_and_others":
                tabs[name] = tabs[name] - {ln, ex}
        return tabs

    bacc.get_activation_tables = patched
    try:
        nc.compile()
    finally:
        bacc.get_activation_tables = orig_fn
    return nc


def _get_module():
    if "nc" not in _cache:
        _cache["nc"] = _build_module()
    return _cache["nc"]


def kernel(x, adj, noise, W, a_src, a_dst, W_out):
    from concourse.bass_utils import run_bass_kernel_spmd

    nc = _get_module()

    bfdt = ml_dtypes.bfloat16
    x = np.asarray(x, dtype=np.float32)
    adj = np.asarray(adj, dtype=np.float32)
    noise = np.asarray(noise, dtype=np.float32)
    W = np.asarray(W, dtype=np.float32)
    a_src = np.asarray(a_src, dtype=np.float32)
    a_dst = np.asarray(a_dst, dtype=np.float32)
    W_out = np.asarray(W_out, dtype=np.float32)

    # fold the per-head score weights: s = (x @ W) @ a_flat / H == x @ (W @ a_flat / H)
    w_src = (W @ a_src.reshape(-1)) / H
    w_dst = (W @ a_dst.reshape(-1)) / H
    # column 0 = dst so the ssd PSUM row read starts at partition 0
    wsd = np.stack([w_dst, w_src], axis=1).astype(bfdt)
    WWo = (W @ W_out).astype(bfdt)
    adj_bf = adj.astype(bfdt)  # exact for 0/1 values
    # w = 1-u encoding keeps full relative precision where it matters (u->1);
    # clamp so Ln(1-w) never sees a zero argument after the bf16 cast.
    wn = np.clip(1.0 - noise, 2.0**-24, 0.99609375).astype(bfdt)

    in_maps = []
    for core in range(N_CORES):
        b, rb = core // 2, core % 2
        rows = slice(rb * RB, (rb + 1) * RB)
        xTb = np.ascontiguousarray(x[b].T).astype(bfdt)  # [IN_F, N]
        in_maps.append({
            "xT": xTb,
            "xTr": np.ascontiguousarray(xTb[:, rows]),
            "adj_s": np.ascontiguousarray(adj_bf[rows, :]),
            "wn_s": np.ascontiguousarray(wn[b, rows, :]),
            "wsd": wsd,
            "WWo": WWo,
        })

    res = run_bass_kernel_spmd(nc, in_maps, list(range(N_CORES)))
    kernel._last_results = res

    out = np.empty((B, N, OUT_F), dtype=np.float32)
    for b in range(B):
        acc = res.results[2 * b]["outT"].astype(np.float32) + \
            res.results[2 * b + 1]["outT"].astype(np.float32)
        out[b] = acc.T
    return out


# revision 17
# speedup vs baseline: 1.8271x; 1.2555x over previous
"""GSAT graph-attention kernel for 8 Trainium2 NeuronCores.

Math (per batch b):
  h = x @ W                                     [N, 512]
  ss[i] = h[i] . a_src_flat / H ; sd[j] = h[j] . a_dst_flat / H
  t[i,j] = (ss[i] + sd[j]) * adj[i,j] + gumbel(noise[b,i,j])
  A1 = softmax_j(t) ; A2 = softmax_j(A1)
  out[b,n] = sum_i A2[i,n] * (h[i] @ W_out)

Sharding: 8 cores = (batch b in 0..3) x (row-half rb in 0..1).  Rows i are
sharded; both softmaxes are along j (within-row), so each core computes its
2048 rows completely and produces a partial output outT summed over its
rows; host adds the two row-half partials per batch.

Engine budget note: the elementwise chain is 3 transcendentals + arith per
element.  ACT runs ~1 elem/cycle dtype-independent; DVE runs 2x on all-bf16
tensor ops.  Two per-block schedules are mixed to balance ACT vs DVE:

  route A (ACT-heavy, 4 ACT passes):
     v  = Ln(1 - w)            [ACT, bf16]        (w = 1-u host-encoded)
     g' = Ln(eps - v)          [ACT, bf16]        (= -gumbel)
     m  = (sd_j + ss_i)*adj    [DVE stt, bf16 2x]
     t  = m - g'               [DVE stt, bf16 2x]
     e1 = Exp(t)        accum rs1   [ACT]
     e2 = Exp(e1/rs1)   accum rs2   [ACT]

  route B (DVE-heavy, 2 ACT passes), using
  exp(scores+gumbel) = (1 + adj*(exp(ss_i)exp(sd_j) - 1)) * 1/(-ln(u)):
     v  = Ln(1 - w)            [ACT, f32 out]
     r  = 1/v                  [DVE reciprocal_approx_fast]  (NEGATIVE)
     w2 = esd_j*ess_i - 1      [DVE ts, bf16 4x]
     p  = w2 * adj             [GPSIMD tt (offload) or DVE tt bf16 2x]
     e1 = (p + 1) * r   accum rs1   [DVE stt]    (e1, rs1 both negative;
     e2 = Exp(e1/rs1)   accum rs2   [ACT]         the sign cancels in e1/rs1)

W_out is folded before aggregation ((A^T H)Wo = A^T(H Wo)); k = x @ (W Wo)
with W Wo folded on the host.  1/rs2 is folded into k.  Both Ln and Exp live
in the 'natural_log_exp_and_others' ACT table set; compile-time table lists
are filtered so the fixpoint pass picks that set (1 table load instead of the
per-block ping-pong between natural_log and exp_and_others).
"""

import os
import sys

for _p in ("/opt/trn_rl_repo",):
    if _p not in sys.path and os.path.isdir(_p):
        sys.path.insert(0, _p)

os.environ.setdefault("MYCRO_LOCAL_CACHE", "1")

import numpy as np
import ml_dtypes

B, N, IN_F, H, OUT_F = 4, 4096, 256, 8, 64
D = H * OUT_F          # 512
RB = N // 2            # 2048 rows per core
NBLK = RB // 128       # 16 row blocks per core
EPS = 1e-9
N_CORES = 8

# Tuning knobs (env overrides are for local experiments only; defaults baked)
ROUTE_A = set(int(x) for x in os.environ.get("KRN_ROUTE_A", "2,5,8,11,14").split(",") if x != "")
GP_TT = os.environ.get("KRN_GP_TT", "0") == "1"   # B-route mask-mult on GPSIMD

_cache = {}


def _build_module():
    import contextlib

    import concourse.bacc as bacc
    import concourse.bass as bass_mod
    import concourse.tile as tile
    from concourse import mybir

    f32 = mybir.dt.float32
    bf16 = mybir.dt.bfloat16
    AF = mybir.ActivationFunctionType
    ALU = mybir.AluOpType

    nc = bacc.Bacc("TRN2", target_bir_lowering=False)

    xT_d = nc.declare_dram_parameter("xT", [IN_F, N], bf16, isOutput=False)
    xTr_d = nc.declare_dram_parameter("xTr", [IN_F, RB], bf16, isOutput=False)
    adj_d = nc.declare_dram_parameter("adj_s", [RB, N], bf16, isOutput=False)
    wn_d = nc.declare_dram_parameter("wn_s", [RB, N], bf16, isOutput=False)
    wsd_d = nc.declare_dram_parameter("wsd", [IN_F, 2], bf16, isOutput=False)
    WWo_d = nc.declare_dram_parameter("WWo", [IN_F, OUT_F], bf16, isOutput=False)
    outT_d = nc.declare_dram_parameter("outT", [OUT_F, N], f32, isOutput=True)

    with tile.TileContext(nc) as tc:
        with contextlib.ExitStack() as ctx:
            pers = ctx.enter_context(tc.tile_pool(name="pers", bufs=1))
            sdb = pers.tile([128, N], bf16)       # raw s_dst broadcast (route A)
            sdbe = pers.tile([128, N], bf16)      # exp(s_dst) broadcast (route B)
            ss_col = pers.tile([128, NBLK], f32)  # ss_col[p, b] = s_src[b*128+p]
            ess_col = pers.tile([128, NBLK], f32)  # exp(ss_col)
            ktil = [pers.tile([128, OUT_F], f32, tag=f"k{ib}", name=f"k{ib}")
                    for ib in range(NBLK)]
            epsb = pers.tile([128, 1], f32)
            nc.vector.memset(epsb, EPS)
            oneb = pers.tile([128, 1], f32)
            nc.vector.memset(oneb, 1.0)

            # streaming pools for the main loop (declared early so the first
            # block's noise/adj DMAs can be issued ahead of the rest of
            # phase 0)
            nzpool = ctx.enter_context(tc.tile_pool(name="nzs", bufs=4))
            adpool = ctx.enter_context(tc.tile_pool(name="ads", bufs=4))
            wpool = ctx.enter_context(tc.tile_pool(name="work", bufs=2))
            vpool = ctx.enter_context(tc.tile_pool(name="vwork", bufs=1))
            epool = ctx.enter_context(tc.tile_pool(name="ework", bufs=3))
            mpool = ctx.enter_context(tc.tile_pool(name="mwork", bufs=1))
            rpool = ctx.enter_context(tc.tile_pool(name="smalls", bufs=4))

            def issue_dma(ib):
                nz = nzpool.tile([128, N], bf16, tag="nz", name=f"nz{ib}")
                nc.sync.dma_start(out=nz, in_=wn_d[ib * 128:(ib + 1) * 128, :])
                ad = adpool.tile([128, N], bf16, tag="ad", name=f"ad{ib}")
                nc.sync.dma_start(out=ad, in_=adj_d[ib * 128:(ib + 1) * 128, :])
                return nz, ad

            def stage_early(ib, nz):
                if ib in ROUTE_A:
                    # v then g' = -gumbel, both in place in bf16
                    nc.scalar.activation(out=nz, in_=nz, func=AF.Ln, bias=oneb, scale=-1.0)
                    nc.scalar.activation(out=nz, in_=nz, func=AF.Ln, bias=epsb, scale=-1.0)
                    return None
                v = vpool.tile([128, N], f32, tag="v", name=f"v{ib}")
                nc.scalar.activation(out=v, in_=nz, func=AF.Ln, bias=oneb, scale=-1.0)
                r = wpool.tile([128, N], f32, tag="r", name=f"r{ib}")
                nc.vector.reciprocal_approx_fast(out=r, in_=v)   # r = 1/v < 0
                return r

            # stage_mid needs phase-0 products (sdbe, ess_col); first issued
            # after phase 0 closes, then one block ahead inside the loop.
            def stage_mid(ib, ad):
                if ib in ROUTE_A:
                    return None
                # w2 = esd_j * ess_i - 1  (ts hits 4x bf16; stt would be 1x)
                w2 = epool.tile([128, N], bf16, tag="ew", name=f"w2{ib}")
                nc.vector.tensor_scalar(out=w2, in0=sdbe,
                                        scalar1=ess_col[:, ib:ib + 1],
                                        scalar2=-1.0,
                                        op0=ALU.mult, op1=ALU.add)
                # p = w2 * adj on GPSIMD (or DVE) one block ahead of use
                eng = nc.gpsimd if GP_TT else nc.vector
                eng.tensor_tensor(out=w2, in0=w2, in1=ad, op=ALU.mult)
                return w2

            # ---------------- phase 0 ----------------
            early_q = {}
            with tc.tile_pool(name="p0", bufs=1) as p0:
                xT2 = [p0.tile([128, N], bf16, tag=f"xT{fc}", name=f"xT{fc}") for fc in range(2)]
                xTr2 = [p0.tile([128, RB], bf16, tag=f"xTr{fc}", name=f"xTr{fc}") for fc in range(2)]
                wsdt = [p0.tile([128, 2], bf16, tag=f"wsd{fc}", name=f"wsdt{fc}") for fc in range(2)]
                sd_rows = p0.tile([1, 2 * N], bf16)  # [raw sd | exp sd] rows
                WWot = [p0.tile([128, OUT_F], bf16, tag=f"WWo{fc}", name=f"WWot{fc}") for fc in range(2)]
                dma_q = {0: issue_dma(0)}
                for fc in range(2):
                    nc.sync.dma_start(out=wsdt[fc], in_=wsd_d[fc * 128:(fc + 1) * 128, :])
                    nc.sync.dma_start(out=xT2[fc], in_=xT_d[fc * 128:(fc + 1) * 128, :])
                dma_q[1] = issue_dma(1)
                for fc in range(2):
                    nc.sync.dma_start(out=xTr2[fc], in_=xTr_d[fc * 128:(fc + 1) * 128, :])
                    nc.sync.dma_start(out=WWot[fc], in_=WWo_d[fc * 128:(fc + 1) * 128, :])

                dma_q[2] = issue_dma(2)
                # ACT can start on block 0 immediately (depends only on nz DMA)
                nz0, ad0 = dma_q.pop(0)
                early_q[0] = (nz0, ad0, stage_early(0, nz0))

                # s_src/s_dst for ALL nodes: ssd[2, n] = wsd^T @ xT
                with tc.tile_pool(name="ps_a", bufs=1, space="PSUM") as ps_a:
                    ssd_ps = [ps_a.tile([2, 512], f32, tag=f"ssd{jc}", name=f"ssd{jc}")
                              for jc in range(8)]
                    for jc in range(8):
                        for fc in range(2):
                            nc.tensor.matmul(ssd_ps[jc], wsdt[fc],
                                             xT2[fc][:, jc * 512:(jc + 1) * 512],
                                             start=(fc == 0), stop=(fc == 1))
                    # rows: 0 = s_dst(all nodes), 1 = s_src(all nodes) [unused]
                    for jc in range(8):
                        sl = slice(jc * 512, (jc + 1) * 512)
                        # raw sd (route A): DVE copy, psum -> sbuf bf16
                        nc.vector.tensor_copy(sd_rows[0:1, sl], ssd_ps[jc][0:1, :])
                        # exp sd (route B): ACT Exp
                        nc.scalar.activation(out=sd_rows[0:1, N + jc * 512:N + (jc + 1) * 512],
                                             in_=ssd_ps[jc][0:1, :], func=AF.Exp)

                # broadcast the two rows down 128 partitions via DRAM scratch
                sd_dram = nc.dram_tensor("sd_scratch", [1, 2 * N], bf16)
                nc.sync.dma_start(out=sd_dram[:], in_=sd_rows)
                raw_ap = sd_dram[0:1, 0:N]
                exp_ap = sd_dram[0:1, N:2 * N]
                raw_b = bass_mod.AP(tensor=raw_ap.tensor, offset=raw_ap.offset,
                                    ap=[[0, 128]] + list(raw_ap.ap)[1:])
                exp_b = bass_mod.AP(tensor=exp_ap.tensor, offset=exp_ap.offset,
                                    ap=[[0, 128]] + list(exp_ap.ap)[1:])
                nc.gpsimd.dma_start(out=sdb, in_=raw_b)
                nc.gpsimd.dma_start(out=sdbe, in_=exp_b)

                # ss_col + k tiles
                with tc.tile_pool(name="ps_b", bufs=2, space="PSUM") as ps_b:
                    sscol_ps = ps_b.tile([128, NBLK], f32, tag="sscol")
                    for ib in range(NBLK):
                        for fc in range(2):
                            nc.tensor.matmul(sscol_ps[:, ib:ib + 1],
                                             xTr2[fc][:, ib * 128:(ib + 1) * 128],
                                             wsdt[fc][:, 1:2],
                                             start=(fc == 0), stop=(fc == 1))
                    nc.vector.tensor_copy(ss_col, sscol_ps)
                    nc.scalar.activation(out=ess_col, in_=ss_col, func=AF.Exp)

                    for ib in range(NBLK):
                        kps = ps_b.tile([128, OUT_F], f32, tag="kps")
                        for fc in range(2):
                            nc.tensor.matmul(kps,
                                             xTr2[fc][:, ib * 128:(ib + 1) * 128],
                                             WWot[fc],
                                             start=(fc == 0), stop=(fc == 1))
                        nc.vector.tensor_copy(ktil[ib], kps)

            # ---------------- main loop ----------------
            aggpool = ctx.enter_context(tc.tile_pool(name="agg", bufs=1, space="PSUM"))
            aggp = [aggpool.tile([OUT_F, 512], f32, tag=f"agg{j}", name=f"agg{j}")
                    for j in range(8)]

            def stage_late(ib, nz, ad, r, w2):
                rs1 = rpool.tile([128, 1], f32, tag="rs1")
                if ib in ROUTE_A:
                    m = mpool.tile([128, N], bf16, tag="m", name=f"m{ib}")
                    # m = (sd_j + ss_i)*1  (two-op ts with the AP scalar in
                    # slot 1 -- the form that measures 4x; AP in slot 2 or
                    # single-op forms measure 4-6x slower)
                    nc.vector.tensor_scalar(out=m, in0=sdb,
                                            scalar1=ss_col[:, ib:ib + 1],
                                            scalar2=1.0,
                                            op0=ALU.add, op1=ALU.mult)
                    # m *= adj  (tt 2x)
                    nc.vector.tensor_tensor(out=m, in0=m, in1=ad, op=ALU.mult)
                    # t = m - g'   (g' in nz; tt 2x)
                    nc.vector.tensor_tensor(out=m, in0=m, in1=nz, op=ALU.subtract)
                    # e1 = exp(t), rs1 = rowsum
                    nc.scalar.activation(out=m, in_=m, func=AF.Exp, accum_out=rs1)
                    e = m
                else:
                    # e1 = (p + 1) * r   (negative), rs1 = rowsum (negative)
                    nc.vector.scalar_tensor_tensor(out=w2, in0=w2, scalar=1.0,
                                                   in1=r, op0=ALU.add, op1=ALU.mult,
                                                   accum_out=rs1)
                    e = w2

                rs1r = rpool.tile([128, 1], f32, tag="rs1r")
                nc.vector.reciprocal(rs1r, rs1)
                # e2 = exp(e1/rs1), rs2 = rowsum(e2)
                rs2 = rpool.tile([128, 1], f32, tag="rs2")
                nc.scalar.activation(out=e, in_=e, func=AF.Exp, scale=rs1r,
                                     accum_out=rs2)
                rs2r = rpool.tile([128, 1], f32, tag="rs2r")
                nc.vector.reciprocal(rs2r, rs2)
                # k~ = k / rs2
                kt = rpool.tile([128, OUT_F], bf16, tag="kt")
                nc.vector.tensor_scalar(out=kt, in0=ktil[ib], scalar1=rs2r,
                                        scalar2=None, op0=ALU.mult)
                # outT += k~^T @ e2 accumulated in PSUM across blocks
                for ns in range(8):
                    nc.tensor.matmul(aggp[ns], kt,
                                     e[:, ns * 512:(ns + 1) * 512],
                                     start=(ib == 0), stop=(ib == NBLK - 1))

            mid_q = {0: stage_mid(0, early_q[0][1]),
                     1: stage_mid(1, dma_q[1][1])}
            for ib in range(NBLK):
                if ib + 3 < NBLK:
                    dma_q[ib + 3] = issue_dma(ib + 3)
                if ib + 1 < NBLK:
                    nz1, ad1 = dma_q.pop(ib + 1)
                    early_q[ib + 1] = (nz1, ad1, stage_early(ib + 1, nz1))
                if ib + 2 < NBLK:
                    mid_q[ib + 2] = stage_mid(ib + 2, dma_q[ib + 2][1])
                nz, ad, r = early_q.pop(ib)
                stage_late(ib, nz, ad, r, mid_q.pop(ib))

            # ---------------- epilogue ----------------
            with tc.tile_pool(name="fin", bufs=1) as fpool:
                outT = fpool.tile([OUT_F, N], f32)
                for ns in range(8):
                    nc.scalar.copy(outT[:, ns * 512:(ns + 1) * 512], aggp[ns])
                nc.sync.dma_start(out=outT_d[:], in_=outT)

    # Compile with table lists filtered so Ln and Exp both resolve to the
    # combined 'natural_log_exp_and_others' set (single ACT_TABLE_LOAD).
    import concourse.hw_specs as hw_specs

    orig_fn = bacc.get_activation_tables

    def patched(arch):
        tabs = dict(orig_fn(arch))
        ln, ex = mybir.ActivationFunctionType.Ln, mybir.ActivationFunctionType.Exp
        for name in list(tabs):
            if name != "natural_log_exp_and_others":
                tabs[name] = tabs[name] - {ln, ex}
        return tabs

    bacc.get_activation_tables = patched
    try:
        nc.compile()
    finally:
        bacc.get_activation_tables = orig_fn
    return nc


def _get_module():
    if "nc" not in _cache:
        _cache["nc"] = _build_module()
    return _cache["nc"]


def kernel(x, adj, noise, W, a_src, a_dst, W_out):
    from concourse.bass_utils import run_bass_kernel_spmd

    nc = _get_module()

    bfdt = ml_dtypes.bfloat16
    x = np.asarray(x, dtype=np.float32)
    adj = np.asarray(adj, dtype=np.float32)
    noise = np.asarray(noise, dtype=np.float32)
    W = np.asarray(W, dtype=np.float32)
    a_src = np.asarray(a_src, dtype=np.float32)
    a_dst = np.asarray(a_dst, dtype=np.float32)
    W_out = np.asarray(W_out, dtype=np.float32)

    # fold the per-head score weights: s = (x @ W) @ a_flat / H == x @ (W @ a_flat / H)
    w_src = (W @ a_src.reshape(-1)) / H
    w_dst = (W @ a_dst.reshape(-1)) / H
    # column 0 = dst so the ssd PSUM row read starts at partition 0
    wsd = np.stack([w_dst, w_src], axis=1).astype(bfdt)
    WWo = (W @ W_out).astype(bfdt)
    adj_bf = adj.astype(bfdt)  # exact for 0/1 values
    # w = 1-u encoding keeps full relative precision where it matters (u->1);
    # clamp so Ln(1-w) never sees a zero argument after the bf16 cast.
    wn = np.clip(1.0 - noise, 2.0**-24, 0.99609375).astype(bfdt)

    in_maps = []
    for core in range(N_CORES):
        b, rb = core // 2, core % 2
        rows = slice(rb * RB, (rb + 1) * RB)
        xTb = np.ascontiguousarray(x[b].T).astype(bfdt)  # [IN_F, N]
        in_maps.append({
            "xT": xTb,
            "xTr": np.ascontiguousarray(xTb[:, rows]),
            "adj_s": np.ascontiguousarray(adj_bf[rows, :]),
            "wn_s": np.ascontiguousarray(wn[b, rows, :]),
            "wsd": wsd,
            "WWo": WWo,
        })

    res = run_bass_kernel_spmd(nc, in_maps, list(range(N_CORES)))
    kernel._last_results = res

    out = np.empty((B, N, OUT_F), dtype=np.float32)
    for b in range(B):
        acc = res.results[2 * b]["outT"].astype(np.float32) + \
            res.results[2 * b + 1]["outT"].astype(np.float32)
        out[b] = acc.T
    return out
